# revision 14
# baseline (speedup 1.0000x reference)
"""Bass/Trainium2 kernel for EnhancedBoundaryDiceLoss (weighted softmax dice).

Contract: kernel(**inputs) takes the FULL inputs (inputs: [8388608, 9] f32,
targets: [8388608] int) and returns the FULL scalar loss (np.float32).

Strategy (data-parallel over the token dim, 8 NeuronCores):
  Each core processes Nc = 1,048,576 tokens. Per 128x512-token tile:
    - ACT: e = exp(x) written group-blocked bf16 [128, NG, 9(+9), G];
      e_sel = exp(x[n, t[n]]) (x_sel is host-gathered -- an indexed copy).
    - DVE: s = per-token sum of the 9 class exps via a pairwise tree over the
      class planes (bf16 2x mode); 1/s via fast-approx reciprocal; boundary
      weights w from targets t and shifted targets tn; q = (w/s) * e_sel;
      one-hot planes oh_c = (t == c) via 9 tensor_scalar is_equal ops.
    - PE: one matmul per G-token group with stationary [r|q|w] columns and
      moving [e-planes | oh-planes] columns; the diagonal blocks of the
      PSUM accumulator collect, per class c:
        A[c] = sum r*e_c     (= sum w*probs_c)
        I[c] = sum q*oh_c    (= sum w*probs_c*onehot_c  -- intersection)
        B[c] = sum w*oh_c    (= sum w*onehot_c)
      accumulated over the whole shard into one PSUM bank (start/stop flags).
  Host: gathers the 8 small [48, 288] grids, reduces the diagonal blocks in
  f64, all-reduces across cores, computes dice = (2I+S)/(A+B+S), loss.
"""

import sys

for _p in ("/opt/trn_rl_repo", "/opt/trn_rl_repo/concourse"):
    if _p not in sys.path:
        sys.path.insert(0, _p)

import numpy as np
import ml_dtypes

import concourse.bass as bass
import concourse.bacc as bacc
import concourse.mybir as mybir
from concourse.tile import TileContext
from concourse.bass_utils import run_bass_kernel_spmd

N_TOKENS = 8388608
C = 9
NCORES = 8
NC = N_TOKENS // NCORES          # 1,048,576 tokens per core
P = 128                          # SBUF partitions
T = 512                          # tokens per partition per tile
NTILES = NC // (P * T)           # 16
G = 16                           # tokens per PE group
NG = T // G                      # 32 PE groups per tile
M = 3 * G                        # 48 PSUM partitions (r/q/w x G)
NCOL = 18 * G                    # 288 PSUM columns (e/oh x 9 classes x G)

B_ID = 1.0
I_ID = 2.0
B_WEIGHT = 3.0
I_END_WEIGHT = 2.5
CONTEXT_WEIGHT = 1.5
SMOOTH = 1e-5

f32 = mybir.dt.float32
bf16 = mybir.dt.bfloat16
Al = mybir.AluOpType
Act = mybir.ActivationFunctionType


def build_bass() -> bass.Bass:
    nc = bacc.Bacc("TRN2", target_bir_lowering=False, debug=False)
    x = nc.dram_tensor("x", [NC, C], f32, kind="ExternalInput")
    xsel = nc.dram_tensor("xsel", [NC], f32, kind="ExternalInput")
    text = nc.dram_tensor("text", [NC + 1], bf16, kind="ExternalInput")
    out = nc.dram_tensor("out", [M, NCOL], f32, kind="ExternalOutput")

    xv = x.rearrange("(n p g l) c -> n p g l c", p=P, g=NG, l=G)
    tv = text[0:NC].rearrange("(n p g l) -> n p g l", p=P, g=NG, l=G)
    tnv = text[1 : NC + 1].rearrange("(n p g l) -> n p g l", p=P, g=NG, l=G)
    xsv = xsel.rearrange("(n p g l) -> n p g l", p=P, g=NG, l=G)

    with TileContext(nc) as tc:
        with (
            tc.tile_pool(name="xin", bufs=3) as xin,
            tc.tile_pool(name="big", bufs=2) as big,
            tc.tile_pool(name="small", bufs=2) as small,
            tc.tile_pool(name="consts", bufs=1) as consts,
            tc.tile_pool(name="acc", bufs=1, space="PSUM") as accp,
        ):
            c15 = consts.tile([P, T], bf16)
            nc.vector.memset(c15, CONTEXT_WEIGHT)
            acc = accp.tile([M, NCOL], f32)
            outsb = consts.tile([M, NCOL], f32)

            for i in range(NTILES):
                xt = xin.tile([P, NG, G, C], f32, tag="xt")
                nc.sync.dma_start(out=xt, in_=xv[i])
                tt = small.tile([P, NG, G], bf16, tag="tt")
                nc.sync.dma_start(out=tt, in_=tv[i])
                tn = small.tile([P, NG, G], bf16, tag="tn")
                nc.sync.dma_start(out=tn, in_=tnv[i])
                xs = small.tile([P, NG, G], f32, tag="xs")
                nc.sync.dma_start(out=xs, in_=xsv[i])

                # group-blocked: per group, 18 planes of G values; planes
                # 0..8 = exp(x), planes 9..17 = one-hot
                eoh = big.tile([P, NG, 18, G], bf16, tag="eoh")
                nc.scalar.activation(
                    out=eoh[:, :, 0:C, :].rearrange("p g c l -> p g l c"),
                    in_=xt,
                    func=Act.Exp,
                )
                es = small.tile([P, NG, G], bf16, tag="es")
                nc.scalar.activation(out=es, in_=xs, func=Act.Exp)

                # s = sum of the 9 exp planes (pairwise tree, bf16 2x mode)
                l1 = small.tile([P, NG, 4, G], bf16, tag="l1")
                nc.vector.tensor_tensor(
                    out=l1,
                    in0=eoh[:, :, 0:4, :],
                    in1=eoh[:, :, 4:8, :],
                    op=Al.add,
                )
                l2 = small.tile([P, NG, 2, G], bf16, tag="l2")
                nc.vector.tensor_tensor(
                    out=l2, in0=l1[:, :, 0:2, :], in1=l1[:, :, 2:4, :], op=Al.add
                )
                s = small.tile([P, T], f32, tag="s")
                sv = s.rearrange("p (g l) -> p g l", l=G)
                nc.vector.tensor_tensor(
                    out=sv, in0=l2[:, :, 0, :], in1=l2[:, :, 1, :], op=Al.add
                )
                s2 = small.tile([P, T], f32, tag="s2")
                nc.vector.tensor_tensor(
                    out=s2.rearrange("p (g l) -> p g l", l=G),
                    in0=sv,
                    in1=eoh[:, :, 8, :],
                    op=Al.add,
                )
                rs = small.tile([P, T], f32, tag="rs")
                nc.vector.reciprocal_approx_fast(out=rs[:], in_=s2[:])

                # boundary weights w
                t2 = tt.rearrange("p g l -> p (g l)")
                tn2 = tn.rearrange("p g l -> p (g l)")
                wa = small.tile([P, T], bf16, tag="wa")
                nc.vector.tensor_scalar(
                    out=wa, in0=t2, scalar1=B_ID, scalar2=B_WEIGHT - 1.0,
                    op0=Al.is_equal, op1=Al.mult,
                )
                i2 = small.tile([P, T], bf16, tag="i2")
                nc.vector.tensor_single_scalar(
                    out=i2, in_=t2, scalar=I_ID, op=Al.is_equal
                )
                n2 = small.tile([P, T], bf16, tag="n2")
                nc.vector.tensor_single_scalar(
                    out=n2, in_=tn2, scalar=I_ID, op=Al.not_equal
                )
                iend = small.tile([P, T], bf16, tag="iend")
                nc.vector.tensor_tensor(out=iend, in0=i2, in1=n2, op=Al.mult)
                w = small.tile([P, T], bf16, tag="w")
                nc.vector.scalar_tensor_tensor(
                    out=w, in0=iend, scalar=I_END_WEIGHT - 1.0, in1=wa,
                    op0=Al.mult, op1=Al.add,
                )
                nc.vector.tensor_single_scalar(
                    out=w, in_=w, scalar=1.0, op=Al.add
                )
                isB = small.tile([P, T], mybir.dt.uint8, tag="isB")
                nc.vector.tensor_single_scalar(
                    out=isB, in_=tn2, scalar=B_ID, op=Al.is_equal
                )
                nc.vector.copy_predicated(out=w, mask=isB, data=c15)

                # r = w / s ; q = r * e_sel
                rbf = small.tile([P, T], bf16, tag="rbf")
                nc.vector.tensor_tensor(out=rbf, in0=w, in1=rs, op=Al.mult)
                q = small.tile([P, T], bf16, tag="q")
                nc.vector.tensor_tensor(
                    out=q, in0=rbf, in1=es.rearrange("p g l -> p (g l)"),
                    op=Al.mult,
                )

                # assemble the stationary [r|q|w] planes (group-blocked)
                rqw = big.tile([P, NG, 3, G], bf16, tag="rqw")
                nc.vector.tensor_copy(
                    out=rqw[:, :, 0, :],
                    in_=rbf.rearrange("p (g l) -> p g l", l=G),
                )
                nc.vector.tensor_copy(
                    out=rqw[:, :, 1, :],
                    in_=q.rearrange("p (g l) -> p g l", l=G),
                )
                nc.vector.tensor_copy(
                    out=rqw[:, :, 2, :],
                    in_=w.rearrange("p (g l) -> p g l", l=G),
                )

                # one-hot planes
                for c in range(C):
                    nc.vector.tensor_single_scalar(
                        out=eoh[:, :, C + c, :], in_=tt, scalar=float(c),
                        op=Al.is_equal,
                    )

                # PE: grouped diagonal matmuls, PSUM-accumulated over the shard
                for g in range(NG):
                    lhsT = rqw[:, g, :, :].rearrange("p f l -> p (f l)")
                    rhs = eoh[:, g, :, :].rearrange("p c l -> p (c l)")
                    nc.tensor.matmul(
                        out=acc[:],
                        lhsT=lhsT,
                        rhs=rhs,
                        start=(i == 0 and g == 0),
                        stop=(i == NTILES - 1 and g == NG - 1),
                    )

            nc.vector.tensor_copy(out=outsb, in_=acc[:])
            nc.sync.dma_start(out=out[:, :], in_=outsb[:])
    nc.compile()
    return nc


_NC_CACHE = None


def _get_bass():
    global _NC_CACHE
    if _NC_CACHE is None:
        _NC_CACHE = build_bass()
    return _NC_CACHE


def _prepare_in_maps(inputs: np.ndarray, targets: np.ndarray):
    inputs = np.asarray(inputs, dtype=np.float32)
    tgt = np.asarray(targets).astype(np.int64)
    xsel_full = np.take_along_axis(inputs, tgt[:, None], axis=1)[:, 0]
    tgt_bf = tgt.astype(ml_dtypes.bfloat16)
    in_maps = []
    for c in range(NCORES):
        lo, hi = c * NC, (c + 1) * NC
        text = np.empty(NC + 1, dtype=ml_dtypes.bfloat16)
        text[:NC] = tgt_bf[lo:hi]
        # pad with the next core's first target; global end pads with I_ID,
        # which reproduces torch/jax semantics for the final token (the
        # I-end check self-suppresses and the context check can't fire).
        text[NC] = tgt_bf[hi] if hi < N_TOKENS else ml_dtypes.bfloat16(I_ID)
        in_maps.append(
            {
                "x": np.ascontiguousarray(inputs[lo:hi]),
                "xsel": np.ascontiguousarray(xsel_full[lo:hi]),
                "text": text,
            }
        )
    return in_maps


def _finish(per_core_outs):
    """Reduce the PSUM grids: diagonal blocks -> A, I, B -> dice loss.

    acc[m, n] with m = f*G + l and n = cc*G + l'; the valid (diagonal)
    entries are l == l'.
    """
    A = np.zeros(C, dtype=np.float64)
    I = np.zeros(C, dtype=np.float64)
    B = np.zeros(C, dtype=np.float64)
    for grid in per_core_outs:
        g64 = np.asarray(grid, dtype=np.float64).reshape(3, G, 18, G)
        # take the l == l' diagonal: result [3, 18, G]
        d = np.einsum("flcl->fcl", g64)
        A += d[0, 0:C, :].sum(axis=1)
        I += d[1, C:18, :].sum(axis=1)
        B += d[2, C:18, :].sum(axis=1)
    denom = A + B
    dice = (2.0 * I + SMOOTH) / (denom + SMOOTH)
    loss = 1.0 - dice.mean()
    return np.float32(loss)


def _install_ntff_shim():
    """The image's antenv lacks axon_hooks; recreate it so trace=True works."""
    import types

    if "antenv.axon_hooks" in sys.modules:
        return
    mod = types.ModuleType("antenv.axon_hooks")
    mod._hook = None
    mod.set_axon_ntff_profile_hook = lambda h: setattr(mod, "_hook", h)
    mod.get_axon_ntff_profile_hook = lambda: mod._hook
    sys.modules["antenv.axon_hooks"] = mod
    try:
        from trn_agent_boot.trn_boot import _ntff_profile_via_ctypes

        hook = _ntff_profile_via_ctypes("/opt/axon/libaxon_pjrt.so")
        if hook is not None:
            mod.set_axon_ntff_profile_hook(hook)
    except Exception as e:  # pragma: no cover - profiling is best-effort
        print(f"ntff shim install failed: {e}", file=sys.stderr)

    # artifact upload needs a bucket this container doesn't have; make it
    # a no-op so the trace path can't die on it.
    import concourse.bass_utils as bu

    _orig_upload = bu.upload_artifacts

    def _safe_upload(tmpdir):
        try:
            return _orig_upload(tmpdir)
        except Exception:
            return tmpdir

    bu.upload_artifacts = _safe_upload


def run(inputs, targets, trace=False):
    if trace:
        try:
            _install_ntff_shim()
        except Exception:
            pass
    nc = _get_bass()
    in_maps = _prepare_in_maps(inputs, targets)
    res = run_bass_kernel_spmd(
        nc, in_maps, core_ids=list(range(NCORES)), trace=trace
    )
    loss = _finish([r["out"] for r in res.results])
    return loss, res


def kernel(inputs, targets):
    loss, _ = run(inputs, targets, trace=False)
    return loss
